# revision 8
# baseline (speedup 1.0000x reference)
"""Trainium2 Bass kernel for 2-layer heterogeneous GNN (EntityClassify).

Math (exact up to fp reassociation):
    segment_sum(X[src] @ W + b, dst) = segment_sum(X[src], dst) @ W + deg ⊗ b
so each core:
  1. scatter-aggregates raw source rows into per-dst-shard accumulators
     (dma_gather with int16 src-bucket indices -> dma_scatter_add),
  2. applies the per-relation linear transforms + deg⊗b (rank-1 matmul)
     + the reference's ReLU chain on its 1/8 dst shard,
  3. AllGathers the combined layer-1 hidden state,
  4. repeats the same edge streams for layer 2 (source = gathered H1).

Scatter-add duplicate safety: within one dma_scatter_add call all dst indices
are unique by construction (per-(round,bucket) streams hold each dst at most
once); calls to the same accumulator are completion-serialized by Tile's WAW
dependency, so the SDMA CCE read-modify-write hazard cannot fire.
"""
import sys
if '/opt/trn_rl_repo' not in sys.path:
    sys.path.insert(0, '/opt/trn_rl_repo')

import numpy as np


def legalize_waits(nc, max_waits: int = 1):
    """This walrus build rejects instructions with >1 sync wait; split excess
    waits onto preceding same-engine NOPs."""
    import bass_rust as _bass_rust
    n_fixed = 0
    for b in nc.main_func.blocks:
        idx = 0
        while idx < len(b.instructions):
            ins = b.instructions[idx]
            si = ins.sync_info
            waits = list(si.on_wait) if si and si.on_wait else []
            if len(waits) <= max_waits:
                idx += 1
                continue
            keep = waits[-max_waits:]
            excess = waits[:-max_waits]
            nops = []
            while excess:
                chunk, excess = excess[:max_waits], excess[max_waits:]
                nop = nc.engines[ins.engine].nop(nofuse=True, hint="waitfix").ins
                nop.sync_info = _bass_rust.SyncInfo(on_wait=chunk, on_update=[])
                nops.append(nop)
            ins.sync_info = _bass_rust.SyncInfo(
                on_wait=keep, on_update=list(si.on_update) if si.on_update else [])
            for b2 in nc.main_func.blocks:
                for n in nops:
                    if n in b2.instructions:
                        b2.instructions.remove(n)
            pos = b.instructions.index(ins)
            for j, n in enumerate(nops):
                b.instructions.insert(pos + j, n)
            n_fixed += 1
            idx = pos + len(nops) + 1
    return n_fixed


# ----------------------------------------------------------------- config ---
class CFG:
    N_PAPER = 200000
    N_AUTHOR = 100000
    E = 500000
    D = 128
    DOUT = 16
    NC = 8
    BUCKET = 32768
    CHUNK = 4096          # max idxs per gather/scatter call
    GROUP = 512           # transform group (rows per matmul free dim)

    def __init__(self, scale=1, bucket=None, chunk=None):
        self.N_PAPER = CFG.N_PAPER // scale
        self.N_AUTHOR = CFG.N_AUTHOR // scale
        self.E = CFG.E // scale
        self.BUCKET = bucket or CFG.BUCKET
        self.CHUNK = chunk or CFG.CHUNK
        self.PP = self.N_PAPER // self.NC          # paper dst shard
        self.PA = self.N_AUTHOR // self.NC         # author dst shard
        self.PPAD = -(-self.PP // self.GROUP) * self.GROUP
        self.APAD = -(-self.PA // self.GROUP) * self.GROUP
        self.PTBL = self.PPAD + 16                 # +trash row at PPAD
        self.ATBL = self.APAD + 16
        self.CAT = self.PP + self.PA               # per-core rows in H1cat
        self.NCAT = self.NC * self.CAT

    def remap_paper(self, p):
        return (p // self.PP) * self.CAT + (p % self.PP)

    def remap_author(self, a):
        return (a // self.PA) * self.CAT + self.PP + (a % self.PA)


# --------------------------------------------------- host index preparation ---
def _rounds_for_core(cfg, src_mapped, dst_rel, n_buckets):
    """Split one core's edges of one relation into rounds.

    Round structure (all rounds are lists of per-bucket (srcrel, dstrel)):
      round 0: within-bucket dst-rank 0 edges   (unique dst per bucket)
      round 1: within-bucket dst-rank 1 edges   (unique dst per bucket)
      round 2+r: remaining edges, r-th edge per dst globally (unique dst)
    Within any (round, bucket) segment every dst appears at most once ->
    every scatter call has unique dst indices.
    """
    bucket = src_mapped // cfg.BUCKET
    srel = src_mapped % cfg.BUCKET
    order = np.lexsort((dst_rel, bucket))
    b_s, s_s, d_s = bucket[order], srel[order], dst_rel[order]
    # rank of each edge within its (bucket, dst) group
    if len(d_s):
        new_grp = np.empty(len(d_s), dtype=bool)
        new_grp[0] = True
        new_grp[1:] = (b_s[1:] != b_s[:-1]) | (d_s[1:] != d_s[:-1])
        grp_id = np.cumsum(new_grp) - 1
        first_pos = np.zeros(grp_id[-1] + 1, dtype=np.int64)
        np.minimum.at(first_pos, grp_id, np.arange(len(d_s)))
        # minimum.at on zeros fails; compute first occurrence properly:
        first_pos = np.full(grp_id[-1] + 1, len(d_s), dtype=np.int64)
        np.minimum.at(first_pos, grp_id, np.arange(len(d_s)))
        rank = np.arange(len(d_s)) - first_pos[grp_id]
    else:
        rank = np.zeros(0, dtype=np.int64)

    rounds = []
    for j in (0, 1):
        m = rank == j
        segs = []
        for b in range(n_buckets):
            mb = m & (b_s == b)
            segs.append((s_s[mb], d_s[mb]))
        rounds.append(segs)
    # tail: rank >= 2, re-ranked globally per dst
    m = rank >= 2
    ts, td, tb = s_s[m], d_s[m], b_s[m]
    if len(td):
        o2 = np.argsort(td, kind='stable')
        td2, ts2, tb2 = td[o2], ts[o2], tb[o2]
        new2 = np.empty(len(td2), dtype=bool)
        new2[0] = True
        new2[1:] = td2[1:] != td2[:-1]
        g2 = np.cumsum(new2) - 1
        fp = np.full(g2[-1] + 1, len(td2), dtype=np.int64)
        np.minimum.at(fp, g2, np.arange(len(td2)))
        r2 = np.arange(len(td2)) - fp[g2]
        for rr in range(int(r2.max()) + 1):
            mm = r2 == rr
            segs = []
            for b in range(n_buckets):
                mb = mm & (tb2 == b)
                segs.append((ts2[mb], td2[mb]))
            rounds.append(segs)
    return rounds


def _wrap16(idx16, nc_groups=8):
    """[n] -> [128, n/16] int16 wrapped (i -> [i%16, i//16]) and replicated."""
    n = idx16.shape[0]
    assert n % 16 == 0
    w = np.ascontiguousarray(idx16.reshape(n // 16, 16).T).astype(np.int16)
    return np.tile(w, (nc_groups, 1))


def prep_relation(cfg, src, dst, dst_shard, n_src_cat, src_map, trash_row):
    """Build per-core gather/scatter idx arrays + call schedule for one
    (relation, layer) pair.

    Returns (schedule, gidx[NC], sidx[NC], deg[NC]) where schedule is a list
    of call lengths (shared across cores; each a multiple of 128, <= CHUNK,
    each annotated with bucket id), and gidx/sidx are int16 wrapped arrays.
    """
    n_buckets = -(-n_src_cat // cfg.BUCKET)
    per_core = []
    max_rounds = 0
    for c in range(cfg.NC):
        lo = c * dst_shard
        sel = (dst >= lo) & (dst < lo + dst_shard)
        sm = src_map(src[sel])
        dr = (dst[sel] - lo).astype(np.int64)
        rounds = _rounds_for_core(cfg, sm, dr, n_buckets)
        per_core.append(rounds)
        max_rounds = max(max_rounds, len(rounds))

    # unify: for each (round, bucket) the padded segment length = max over cores
    seg_lens = []
    for r in range(max_rounds):
        row = []
        for b in range(n_buckets):
            m = 0
            for c in range(cfg.NC):
                if r < len(per_core[c]):
                    m = max(m, len(per_core[c][r][b][0]))
            row.append(-(-max(m, 0) // 128) * 128 if m else 0)
        seg_lens.append(row)

    # schedule: calls of <= CHUNK per (round, bucket) segment
    schedule = []   # (round, bucket, call_len)
    for r in range(max_rounds):
        for b in range(n_buckets):
            L = seg_lens[r][b]
            off = 0
            while off < L:
                n = min(cfg.CHUNK, L - off)
                schedule.append((r, b, n))
                off += n

    total = sum(n for _, _, n in schedule)
    gidx_all, sidx_all, deg_all = [], [], []
    for c in range(cfg.NC):
        g = np.zeros(total, dtype=np.int64)
        s = np.full(total, trash_row, dtype=np.int64)
        off = 0
        # iterate schedule in order, tracking consumed position per (r,b)
        consumed = {}
        for (r, b, n) in schedule:
            start = consumed.get((r, b), 0)
            if r < len(per_core[c]):
                ss, dd = per_core[c][r][b]
            else:
                ss, dd = np.zeros(0, np.int64), np.zeros(0, np.int64)
            take = min(n, max(0, len(ss) - start))
            if take:
                g[off:off + take] = ss[start:start + take]
                s[off:off + take] = dd[start:start + take]
            consumed[(r, b)] = start + take
            off += n
        assert off == total
        gidx_all.append(_wrap16(g))
        sidx_all.append(_wrap16(s))
        lo = c * dst_shard
        sel = (dst >= lo) & (dst < lo + dst_shard)
        deg = np.bincount(dst[sel] - lo, minlength=dst_shard).astype(np.float32)
        deg_all.append(deg)
    return schedule, gidx_all, sidx_all, deg_all


# ----------------------------------------------------------- bass builder ---
def build_bass(cfg, schedules):
    """schedules: dict (layer, rel) -> schedule list [(round, bucket, n)]."""
    import concourse.bass as bass
    import concourse.tile as tile
    from concourse import mybir, library_config
    from concourse.masks import make_identity

    D, DOUT, G = cfg.D, cfg.DOUT, cfg.GROUP
    f32, i16 = mybir.dt.float32, mybir.dt.int16

    nc = bass.Bass(dynamic_dma_scratch_size=32768)

    # ---- I/O tensors
    xp = nc.dram_tensor("xp", [cfg.N_PAPER, D], f32, kind="ExternalInput")
    xa = nc.dram_tensor("xa", [cfg.N_AUTHOR, D], f32, kind="ExternalInput")
    Ws = {}
    for et in ("cites", "writtenby", "writes"):
        Ws[(1, et, 'W')] = nc.dram_tensor(f"W1_{et}", [D, D], f32, kind="ExternalInput")
        Ws[(1, et, 'b')] = nc.dram_tensor(f"b1_{et}", [1, D], f32, kind="ExternalInput")
        Ws[(2, et, 'W')] = nc.dram_tensor(f"W2_{et}", [D, DOUT], f32, kind="ExternalInput")
        Ws[(2, et, 'b')] = nc.dram_tensor(f"b2_{et}", [1, DOUT], f32, kind="ExternalInput")
    idx_t = {}
    for (l, r), sched in schedules.items():
        total = sum(n for _, _, n in sched)
        idx_t[(l, r, 'g')] = nc.dram_tensor(f"gi_{l}_{r}", [128, total // 16], i16, kind="ExternalInput")
        idx_t[(l, r, 's')] = nc.dram_tensor(f"si_{l}_{r}", [128, total // 16], i16, kind="ExternalInput")
    degs = {
        'cites': nc.dram_tensor("deg_cites", [1, cfg.PPAD], f32, kind="ExternalInput"),
        'writes': nc.dram_tensor("deg_writes", [1, cfg.PPAD], f32, kind="ExternalInput"),
        'writtenby': nc.dram_tensor("deg_writtenby", [1, cfg.APAD], f32, kind="ExternalInput"),
    }
    out_p = nc.dram_tensor("out_p", [cfg.PP, DOUT], f32, kind="ExternalOutput")
    out_a = nc.dram_tensor("out_a", [cfg.PA, DOUT], f32, kind="ExternalOutput")

    with tile.TileContext(nc) as tc:
        with (
            tc.tile_pool(name="const", bufs=1) as cpool,
            tc.tile_pool(name="stream", bufs=3) as spool,
            tc.tile_pool(name="xform", bufs=2) as xpool,
            tc.tile_pool(name="psum", bufs=2, space="PSUM") as ppool,
            tc.tile_pool(name="dram", bufs=1, space="DRAM") as dpool,
        ):
            nc.gpsimd.load_library(library_config.mlp)
            nreg = {}
            for n in sorted({n for sched in schedules.values() for _, _, n in sched}):
                r = nc.alloc_register(mybir.EngineType.Pool, f"n{n}")
                nc.gpsimd.reg_mov(r, n)
                nreg[n] = r

            # ---- internal DRAM
            agg = {}
            for l_ in (1, 2):
                for et_ in ('cites', 'writes', 'writtenby'):
                    rows_ = cfg.ATBL if et_ == 'writtenby' else cfg.PTBL
                    agg[(l_, et_)] = dpool.tile([rows_, D], f32, tag=f"agg{l_}{et_}", name=f"agg{l_}{et_}")
            h1_in = dpool.tile([cfg.CAT, D], f32)       # AG input (this core's shard)
            h1_cat = dpool.tile([cfg.NCAT, D], f32)     # AG output (all cores)

            # ---- zero the accumulators
            z = cpool.tile([128, 2048], f32)
            nc.gpsimd.memset(z[:], 0.0)
            for t in agg.values():
                rows = t.shape[0]
                # zero in [128, 2048] blocks over a flat view
                flat = rows * D
                nblk = flat // (128 * 2048)
                v = t[:].rearrange("r c -> (r c)")
                for i in range(nblk):
                    nc.sync.dma_start(
                        out=v[i * 128 * 2048:(i + 1) * 128 * 2048].rearrange("(p f) -> p f", p=128),
                        in_=z[:])
                rem = flat - nblk * 128 * 2048
                if rem:
                    assert rem % 128 == 0
                    nc.sync.dma_start(
                        out=v[nblk * 128 * 2048:].rearrange("(p f) -> p f", p=128),
                        in_=z[:, :rem // 128])

            # ---- constants for transforms
            ident = cpool.tile([128, 128], f32)
            make_identity(nc, ident[:])
            w_sb, b_sb = {}, {}
            for (l, et, kind), t in Ws.items():
                if kind == 'W':
                    w_sb[(l, et)] = cpool.tile([D, t.shape[1]], f32, tag=f"w{l}{et}", name=f"w{l}{et}")
                    nc.sync.dma_start(out=w_sb[(l, et)][:], in_=t[:])
                else:
                    b_sb[(l, et)] = cpool.tile([1, t.shape[1]], f32, tag=f"b{l}{et}", name=f"b{l}{et}")
                    nc.sync.dma_start(out=b_sb[(l, et)][:], in_=t[:])

            # ---- gather/scatter streams
            def run_streams(layer, rel, src_tensor):
                sched = schedules[(layer, rel)]
                gi, si = idx_t[(layer, rel, 'g')], idx_t[(layer, rel, 's')]
                out_t = agg[(layer, rel)]
                n_src = src_tensor.shape[0]
                off = 0
                for (r, b, n) in sched:
                    gidx = spool.tile([128, cfg.CHUNK // 16], i16, tag="gidx")
                    sidx = spool.tile([128, cfg.CHUNK // 16], i16, tag="sidx")
                    nc.sync.dma_start(out=gidx[:, :n // 16], in_=gi[:, off // 16:(off + n) // 16])
                    nc.sync.dma_start(out=sidx[:, :n // 16], in_=si[:, off // 16:(off + n) // 16])
                    gt = spool.tile([128, cfg.CHUNK // 128, D], f32, tag="gath")
                    base = b * cfg.BUCKET
                    bsz = min(cfg.BUCKET, n_src - base)
                    nc.gpsimd.dma_gather(
                        out_ap=gt[:, :n // 128, :],
                        in_ap=src_tensor[base:base + bsz, :],
                        idxs_ap=gidx[:, :n // 16],
                        num_idxs=n, num_idxs_reg=nreg[n], elem_size=D,
                        single_packet=False)
                    nc.gpsimd.dma_scatter_add(
                        out_ap=out_t[:],
                        in_ap=gt[:, :n // 128, :],
                        idxs_ap=sidx[:, :n // 16],
                        num_idxs=n, num_idxs_reg=nreg[n], elem_size=D,
                        single_packet=False)
                    off += n

            # ---- transform helpers
            def transform(layer, n_groups, rels, dst_store):
                """rels: list of (relation, ) applied in the reference's relu
                chain order; dst_store(g, tile_sb, rows) stores [128,k,dout]."""
                dout = D if layer == 1 else DOUT
                for g in range(n_groups):
                    ats = {}
                    for et in rels:
                        src = agg[(layer, et)]
                        ld = xpool.tile([128, 4, D], f32, tag="ld")
                        nc.sync.dma_start(
                            out=ld[:],
                            in_=src[g * G:(g + 1) * G, :].rearrange("(j p) f -> p j f", p=128))
                        pt = ppool.tile([128, G], f32, tag="pt", space="PSUM")
                        for j in range(4):
                            nc.tensor.transpose(
                                out=pt[:, j * 128:(j + 1) * 128],
                                in_=ld[:, j, :], identity=ident[:])
                        at = xpool.tile([128, G], f32, tag=f"at{et}")
                        nc.vector.tensor_copy(out=at[:], in_=pt[:])
                        ats[et] = at
                    pres = {}
                    for ei, et in enumerate(rels):
                        degt = xpool.tile([1, G], f32, tag=f"deg{ei}")
                        nc.sync.dma_start(out=degt[:], in_=degs[et][:, g * G:(g + 1) * G])
                        po = ppool.tile([dout, G], f32, tag=f"po{ei}", space="PSUM")
                        nc.tensor.matmul(out=po[:], lhsT=w_sb[(layer, et)][:],
                                         rhs=ats[et][:], start=True, stop=False)
                        nc.tensor.matmul(out=po[:], lhsT=b_sb[(layer, et)][:],
                                         rhs=degt[:],
                                         start=False, stop=True)
                        pres[et] = po
                    from concourse import mybir as mb
                    relu = mb.ActivationFunctionType.Relu
                    t1 = xpool.tile([dout, G], f32, tag="t1")
                    nc.scalar.activation(out=t1[:], in_=pres[rels[0]][:], func=relu)
                    if len(rels) == 2:
                        t2 = xpool.tile([dout, G], f32, tag="t2")
                        nc.vector.tensor_add(out=t2[:], in0=t1[:], in1=pres[rels[1]][:])
                        nc.scalar.activation(out=t1[:], in_=t2[:], func=relu)
                    # transpose back to row-major [G, dout]
                    ot = xpool.tile([128, 4, dout], f32, tag="ot")
                    for j in range(4):
                        pb = ppool.tile([128, dout], f32, tag="pb", space="PSUM")
                        nc.tensor.transpose(
                            out=pb[:], in_=t1[:, j * 128:(j + 1) * 128],
                            identity=ident[:dout, :dout] if dout < 128 else ident[:])
                        nc.vector.tensor_copy(out=ot[:, j, :], in_=pb[:])
                    dst_store(g, ot)

            # ================= layer 1 =================
            run_streams(1, 'cites', xp)
            run_streams(1, 'writes', xa)
            run_streams(1, 'writtenby', xp)

            def make_store(dst_t, base, limit):
                def store(g, ot):
                    lo = g * G
                    hi = min(lo + G, limit)
                    if hi <= lo:
                        return
                    if hi - lo == G:
                        nc.sync.dma_start(
                            out=dst_t[base + lo:base + hi, :].rearrange(
                                "(j p) f -> p j f", p=128),
                            in_=ot[:])
                    else:
                        for j in range(4):
                            r0 = lo + j * 128
                            r1 = min(r0 + 128, hi)
                            if r1 <= r0:
                                break
                            nc.sync.dma_start(
                                out=dst_t[base + r0:base + r1, :],
                                in_=ot[:r1 - r0, j, :])
                return store

            store_h1p = make_store(h1_in, 0, cfg.PP)
            store_h1a = make_store(h1_in, cfg.PP, cfg.PA)

            transform(1, cfg.PPAD // G, ['cites', 'writes'], store_h1p)
            transform(1, cfg.APAD // G, ['writtenby'], store_h1a)

            # ================= allgather =================
            nc.gpsimd.collective_compute(
                "AllGather", bass.mybir.AluOpType.bypass,
                replica_groups=[list(range(cfg.NC))],
                ins=[h1_in.opt()], outs=[h1_cat.opt()])

            # ================= layer 2 =================
            run_streams(2, 'cites', h1_cat)
            run_streams(2, 'writes', h1_cat)
            run_streams(2, 'writtenby', h1_cat)

            store_outp = make_store(out_p, 0, cfg.PP)
            store_outa = make_store(out_a, 0, cfg.PA)

            transform(2, cfg.PPAD // G, ['cites', 'writes'], store_outp)
            transform(2, cfg.APAD // G, ['writtenby'], store_outa)

    from concourse.library_overlay import lower_extended_insts
    lower_extended_insts(nc)
    legalize_waits(nc, 1)
    return nc


# --------------------------------------------------------------- frontend ---
def _prepare(cfg, inputs):
    cs, cd = np.asarray(inputs['cites_src']), np.asarray(inputs['cites_dst'])
    ws, wd = np.asarray(inputs['writtenby_src']), np.asarray(inputs['writtenby_dst'])
    rs, rd = np.asarray(inputs['writes_src']), np.asarray(inputs['writes_dst'])

    schedules, gidx, sidx, degv = {}, {}, {}, {}
    specs = {
        (1, 'cites'): (cs, cd, cfg.PP, cfg.N_PAPER, lambda x: x.astype(np.int64), cfg.PPAD),
        (1, 'writtenby'): (ws, wd, cfg.PA, cfg.N_PAPER, lambda x: x.astype(np.int64), cfg.APAD),
        (1, 'writes'): (rs, rd, cfg.PP, cfg.N_AUTHOR, lambda x: x.astype(np.int64), cfg.PPAD),
        (2, 'cites'): (cs, cd, cfg.PP, cfg.NCAT, cfg.remap_paper, cfg.PPAD),
        (2, 'writtenby'): (ws, wd, cfg.PA, cfg.NCAT, cfg.remap_paper, cfg.APAD),
        (2, 'writes'): (rs, rd, cfg.PP, cfg.NCAT, cfg.remap_author, cfg.PPAD),
    }
    for key, (s, d, shard, nsrc, smap, trash) in specs.items():
        sched, g, si, dg = prep_relation(cfg, s.astype(np.int64), d.astype(np.int64),
                                         shard, nsrc, smap, trash)
        schedules[key] = sched
        gidx[key], sidx[key], degv[key] = g, si, dg
    return schedules, gidx, sidx, degv


def _in_maps(cfg, inputs, gidx, sidx, degv):
    maps = []
    for c in range(cfg.NC):
        m = {
            "xp": np.asarray(inputs['embed_paper'], dtype=np.float32),
            "xa": np.asarray(inputs['embed_author'], dtype=np.float32),
        }
        for et in ("cites", "writtenby", "writes"):
            m[f"W1_{et}"] = np.asarray(inputs[f'W1_{et}'], np.float32)
            m[f"b1_{et}"] = np.asarray(inputs[f'b1_{et}'], np.float32).reshape(1, -1)
            m[f"W2_{et}"] = np.asarray(inputs[f'W2_{et}'], np.float32)
            m[f"b2_{et}"] = np.asarray(inputs[f'b2_{et}'], np.float32).reshape(1, -1)
        for (l, r) in gidx:
            m[f"gi_{l}_{r}"] = gidx[(l, r)][c]
            m[f"si_{l}_{r}"] = sidx[(l, r)][c]
        pad = {'cites': cfg.PPAD, 'writes': cfg.PPAD, 'writtenby': cfg.APAD}
        for et in pad:
            dv = degv[(1, et)][c]
            m[f"deg_{et}"] = np.pad(dv, (0, pad[et] - len(dv))).reshape(1, -1).astype(np.float32)
        maps.append(m)
    return maps


_BUILD_CACHE = {}


def run(inputs, cfg=None, trace=False, sim=False):
    cfg = cfg or CFG()
    schedules, gidx, sidx, degv = _prepare(cfg, inputs)
    key = tuple(sorted((k, tuple(v)) for k, v in schedules.items()))
    if key not in _BUILD_CACHE:
        _BUILD_CACHE[key] = build_bass(cfg, schedules)
    nc = _BUILD_CACHE[key]
    maps = _in_maps(cfg, inputs, gidx, sidx, degv)

    if sim:
        from concourse.bass_interp import MultiCoreSim
        msim = MultiCoreSim(nc, num_cores=cfg.NC, require_finite=False, require_nnan=False)
        sims = list(msim.cores.values())
        for c, s in enumerate(sims):
            for k, v in maps[c].items():
                s.tensor(k)[:] = v
        msim.simulate(check_with_hw=False)
        results = [{k: np.array(s.tensor(k)) for k in ("out_p", "out_a")} for s in sims]
        exec_ns = None
    else:
        from concourse.bass_utils import run_bass_kernel_spmd
        res = run_bass_kernel_spmd(nc, maps, core_ids=list(range(cfg.NC)), trace=trace)
        results = res.results
        exec_ns = res.exec_time_ns

    h2p = np.concatenate([results[c]["out_p"] for c in range(cfg.NC)], axis=0)
    h2a = np.concatenate([results[c]["out_a"] for c in range(cfg.NC)], axis=0)
    return (h2p, h2a), exec_ns


def kernel(**inputs):
    (h2p, h2a), _ = run(inputs)
    return h2p, h2a
